# revision 13
# baseline (speedup 1.0000x reference)
"""DeltaRule (diagonal-state linear attention) Bass kernel for 8 TRN2 cores.

Problem: nn_DeltaRule_20194936225992
  B=4, S=2048, H_DIM=1024, N_HEADS=16, HEAD_DIM=64.
  q/k/v/b projections, phi = elu+1, per-(b,h,d) scalar linear recurrence
      s_t = (1 - b_t*pk_t^2) * s_{t-1} + b_t*v_t*pk_t ;  y_t = s_t * pq_t
  out = y @ Wo.T + bo

Sharding: core = (batch b, head-group hg) with hg covering 8 heads.
Each core computes its partial O-projection (contraction over its 512
lanes); host sums the two head-group partials per batch, transposes
[o,t] -> [t,o] and adds bo.

On-device layout: lanes (h*64+d) on partitions, time on free dim.  The
recurrence runs as a hardware `tensor_tensor_scan` per [128,TC] tile,
chained across time chunks via the last column of the previous s.

Engine plan (per lane-tile, per chunk):
  PE:  8 warmup matmuls on junk data (HAM un-throttle during the input
       DMA), Wq/Wk/Wv projections (weights stationary, x.T moving),
       O-projection pipelined ONE CHUNK BEHIND the q/k/v projections so
       the PE never waits for the elementwise chain.
  ACT: relu(x+b), relu(-x-b), exp(-r) pairs for phi (elu+1 computed as
       exp(min(x,0)) + max(x,0)), a = 1 - g affine, PSUM->SBUF O copies
       (bf16 out).  Single act table set, no swaps.
  DVE: pk/pq assembly adds, w = pk*b, g = pk*w, c = (v+bv)*w via
       scalar_tensor_tensor (folds the v bias; no ones-matmul),
       y = s*pq, and the scan itself.
  The sigmoid gate b is computed on the host (0.4% of total FLOPs) and
  DMA'd pre-broadcast per lane.

All matmul operands use IN_DT (bfloat16/float16); everything else
(phi, gates, the scan itself) is fp32.
"""

import os
import sys

for _p in ("/opt/trn_rl_repo", os.path.expanduser("~/.axon_site/_ro/trn_rl_repo")):
    if os.path.isdir(_p) and _p not in sys.path:
        sys.path.insert(0, _p)

import numpy as np  # noqa: E402

import concourse.bass as bass  # noqa: E402
import concourse.tile as tile  # noqa: E402
from concourse import bacc, mybir  # noqa: E402
from concourse.bass import ts  # noqa: E402
from concourse.bass_utils import run_bass_kernel_spmd  # noqa: E402

# problem constants (hardcoded per task rules)
B, S, H_DIM, N_HEADS, HEAD_DIM = 4, 2048, 1024, 16, 64
P = 128
NCORES = 8
HG = 2                      # head groups
J = 512                     # lanes per core  (8 heads * 64)
JT = J // P                 # 4 j-tiles
DT = H_DIM // P             # 8 contraction tiles
HPC = N_HEADS // HG         # 8 heads per core

# matmul-operand dtype: "bfloat16" | "float16" | "float32r" | "float32"
IN_DT_NAME = os.environ.get("DELTA_IN_DT", "float16")
N_WARMUP = int(os.environ.get("DELTA_WARMUP", "8"))
WG_GPS = os.environ.get("DELTA_WG_GPS", "0") != "0"   # w,g on GpSimd
TAIL_GPS = os.environ.get("DELTA_TAIL_GPS", "0") != "0"  # y on GpSimd

F32 = mybir.dt.float32
AF = mybir.ActivationFunctionType


def _tc(in_dt_name):
    return 512 if in_dt_name in ("bfloat16", "float16") else 256


def build_nc(in_dt_name=None):
    if in_dt_name is None:
        in_dt_name = IN_DT_NAME
    in_dt = getattr(mybir.dt, in_dt_name)
    TC = _tc(in_dt_name)
    NCH = S // TC

    nc = bacc.Bacc(trn_type="TRN2", target_bir_lowering=False, debug=False)

    # per-core inputs; x tensors host-packed as [p, chunk, dt, t_in_chunk]
    xq = nc.dram_tensor("xq", [P, NCH, DT, TC], in_dt, kind="ExternalInput").ap()
    xk = nc.dram_tensor("xk", [P, NCH, DT, TC], in_dt, kind="ExternalInput").ap()
    xv = nc.dram_tensor("xv", [P, NCH, DT, TC], in_dt, kind="ExternalInput").ap()
    bbb = nc.dram_tensor("bbb", [P, NCH, JT, TC], in_dt, kind="ExternalInput").ap()
    wq = nc.dram_tensor("wq", [H_DIM, J], in_dt, kind="ExternalInput").ap()
    wk = nc.dram_tensor("wk", [H_DIM, J], in_dt, kind="ExternalInput").ap()
    wv = nc.dram_tensor("wv", [H_DIM, J], in_dt, kind="ExternalInput").ap()
    wo = nc.dram_tensor("wo", [J, H_DIM], in_dt, kind="ExternalInput").ap()
    bq = nc.dram_tensor("bq", [P, JT], F32, kind="ExternalInput").ap()
    bk = nc.dram_tensor("bk", [P, JT], F32, kind="ExternalInput").ap()
    bv = nc.dram_tensor("bv", [P, JT], F32, kind="ExternalInput").ap()
    out = nc.dram_tensor("out", [H_DIM, S], in_dt, kind="ExternalOutput").ap()

    from contextlib import ExitStack

    wk_r = wk.rearrange("(dt p) j -> p dt j", p=P)
    wv_r = wv.rearrange("(dt p) j -> p dt j", p=P)
    wq_r = wq.rearrange("(dt p) j -> p dt j", p=P)

    with tile.TileContext(nc) as tcx, ExitStack() as ctx:
        wpool = ctx.enter_context(tcx.tile_pool(name="weights", bufs=1))
        xpool = ctx.enter_context(tcx.tile_pool(name="xin", bufs=2))
        ipool = ctx.enter_context(tcx.tile_pool(name="inter", bufs=2))
        spool = ctx.enter_context(tcx.tile_pool(name="scan", bufs=2))
        opool = ctx.enter_context(tcx.tile_pool(name="osb", bufs=4))
        pproj = ctx.enter_context(tcx.tile_pool(name="pproj", bufs=6, space="PSUM"))
        po = ctx.enter_context(tcx.tile_pool(name="po", bufs=2, space="PSUM"))

        # --- persistent weights / constants ---
        wq_sb = wpool.tile([P, DT, J], in_dt, tag="wq")
        wk_sb = wpool.tile([P, DT, J], in_dt, tag="wk")
        wv_sb = wpool.tile([P, DT, J], in_dt, tag="wv")
        wo_sb = wpool.tile([P, JT, H_DIM], in_dt, tag="wo")
        bq_sb = wpool.tile([P, JT], F32, tag="bq")
        bk_sb = wpool.tile([P, JT], F32, tag="bk")
        bv_sb = wpool.tile([P, JT], F32, tag="bv")
        wu_sb = wpool.tile([P, TC], in_dt, tag="wu")

        M = mybir.AluOpType

        # --- PE warmup: junk matmuls with no DMA dependency keep the PE
        # busy through the input-DMA head so the HAM clock gate opens
        # (~3.4us of activity) before the first real projection.
        nc.vector.memset(wu_sb[:], 0.0)
        if N_WARMUP:
            pw = po.tile([P, TC], F32, tag="po")
            for i in range(N_WARMUP):
                nc.tensor.matmul(out=pw[:], lhsT=wu_sb[:, 0:P], rhs=wu_sb[:],
                                 start=(i == 0), stop=(i == N_WARMUP - 1))

        s_prev = [None] * JT   # last-chunk scan state tile per lane-tile
        y_prev = None          # previous chunk's y tiles (O-proj pipeline)

        def emit_oproj(c, y_t, split_copies=False):
            """O projection for chunk c: out[o, t] += wo[j, o] * y[j, t]."""
            for ot in range(DT):
                pso = po.tile([P, TC], F32, tag="po")
                for lt in range(JT):
                    nc.tensor.matmul(
                        out=pso[:], lhsT=wo_sb[:, lt, ts(ot, P)], rhs=y_t[lt][:],
                        start=(lt == 0), stop=(lt == JT - 1),
                    )
                o_sb = opool.tile([P, TC], in_dt, tag="osb")
                if split_copies and ot % 2:
                    nc.vector.tensor_copy(o_sb[:], pso[:])
                else:
                    nc.scalar.copy(out=o_sb[:], in_=pso[:])
                nc.sync.dma_start(out=out[ts(ot, P), ts(c, TC)], in_=o_sb[:])

        def alloc_x():
            return (xpool.tile([P, DT, TC], in_dt, tag="xk", name="xk_c"),
                    xpool.tile([P, DT, TC], in_dt, tag="xv", name="xv_c"),
                    xpool.tile([P, DT, TC], in_dt, tag="xq", name="xq_c"),
                    xpool.tile([P, JT, TC], in_dt, tag="bbb", name="bb_c"))

        def emit_x_dmas(c, tiles, eng):
            xk_c, xv_c, xq_c, bb_c = tiles
            eng.dma_start(out=xk_c[:], in_=xk[:, c, :, :])
            eng.dma_start(out=xv_c[:], in_=xv[:, c, :, :])
            eng.dma_start(out=bb_c[:], in_=bbb[:, c, :, :])
            eng.dma_start(out=xq_c[:], in_=xq[:, c, :, :])

        # Prologue DMAs, need-ordered (the PE consumes k-phase, then
        # v-phase, then q-phase).  One whole-tensor DMA each: SWDGE
        # descriptor generation serializes at ~0.6us per dma_start on the
        # issuing engine, so descriptor COUNT in the head is what matters.
        # Chunk-1 x and wo go through the scalar engine's parallel HWDGE
        # queue so their descriptors don't delay chunk-0's.
        x_tiles = alloc_x()
        xk_c, xv_c, xq_c, bb_c = x_tiles
        nc.sync.dma_start(out=wk_sb[:], in_=wk_r)
        nc.sync.dma_start(out=xk_c[:], in_=xk[:, 0, :, :])
        nc.sync.dma_start(out=bk_sb[:], in_=bk)
        nc.sync.dma_start(out=bv_sb[:], in_=bv)
        nc.sync.dma_start(out=bq_sb[:], in_=bq)
        nc.sync.dma_start(out=wv_sb[:], in_=wv_r)
        nc.sync.dma_start(out=xv_c[:], in_=xv[:, 0, :, :])
        nc.sync.dma_start(out=bb_c[:], in_=bbb[:, 0, :, :])
        nc.sync.dma_start(out=wq_sb[:], in_=wq_r)
        nc.sync.dma_start(out=xq_c[:], in_=xq[:, 0, :, :])
        x_next = alloc_x()
        emit_x_dmas(1, x_next, nc.scalar)
        nc.scalar.dma_start(out=wo_sb[:],
                            in_=wo.rearrange("(jt p) o -> p jt o", p=P))

        for c in range(NCH):
            xk_c, xv_c, xq_c, bb_c = x_tiles
            if c + 2 < NCH:
                nxt = alloc_x()
                emit_x_dmas(c + 2, nxt, nc.sync)
            else:
                nxt = None

            # ---- k phase: all lane-tiles' k projections, then phi(k) ----
            # phi(z) = max(z,0) + exp(min(z,0)) = relu(z) + min(exp(z), 1)
            psk_t, rk_t, ek_t = [], [], []
            for lt in range(JT):
                jsl = ts(lt, P)
                psk = pproj.tile([P, TC], F32, tag="proj")
                for d in range(DT):
                    nc.tensor.matmul(
                        out=psk[:], lhsT=wk_sb[:, d, jsl], rhs=xk_c[:, d, :],
                        start=(d == 0), stop=(d == DT - 1),
                    )
                psk_t.append(psk)
            for lt in range(JT):
                rk = ipool.tile([P, TC], in_dt, tag="rpos")
                nc.scalar.activation(out=rk[:], in_=psk_t[lt][:], func=AF.Relu,
                                     bias=bk_sb[:, lt:lt + 1])
                ek = ipool.tile([P, TC], in_dt, tag="ex")
                nc.scalar.activation(out=ek[:], in_=psk_t[lt][:], func=AF.Exp,
                                     bias=bk_sb[:, lt:lt + 1])
                rk_t.append(rk)
                ek_t.append(ek)

            # ---- v phase: v projections; gate/decay chain + scan ----
            s_t = []
            for lt in range(JT):
                jsl = ts(lt, P)
                psv = pproj.tile([P, TC], F32, tag="proj")
                for d in range(DT):
                    nc.tensor.matmul(
                        out=psv[:], lhsT=wv_sb[:, d, jsl], rhs=xv_c[:, d, :],
                        start=(d == 0), stop=(d == DT - 1),
                    )
                pk = ipool.tile([P, TC], in_dt, tag="pk")
                nc.vector.scalar_tensor_tensor(
                    out=pk[:], in0=ek_t[lt][:], scalar=1.0, in1=rk_t[lt][:],
                    op0=M.min, op1=M.add)
                w = ipool.tile([P, TC], in_dt, tag="w")
                nc.vector.tensor_tensor(out=w[:], in0=pk[:], in1=bb_c[:, lt, :], op=M.mult)
                g = ipool.tile([P, TC], in_dt, tag="g")
                nc.vector.tensor_tensor(out=g[:], in0=pk[:], in1=w[:], op=M.mult)
                a = ipool.tile([P, TC], in_dt, tag="a")
                nc.vector.tensor_scalar(out=a[:], in0=g[:], scalar1=-1.0,
                                        scalar2=1.0, op0=M.mult, op1=M.add)
                cc = ipool.tile([P, TC], in_dt, tag="cc")
                nc.vector.scalar_tensor_tensor(
                    out=cc[:], in0=psv[:], scalar=bv_sb[:, lt:lt + 1], in1=w[:],
                    op0=M.add, op1=M.mult)
                # fp32 state inside the scan; operands/output 16-bit
                s_new = spool.tile([P, TC], in_dt, tag=f"s{lt}")
                init = 0.0 if c == 0 else s_prev[lt][:, TC - 1:TC]
                nc.vector.tensor_tensor_scan(
                    out=s_new[:], data0=a[:], data1=cc[:], initial=init,
                    op0=M.mult, op1=M.add,
                )
                s_prev[lt] = s_new
                s_t.append(s_new)

            # ---- q phase: q projections; phi(q); y = s * pq ----
            psq_t = []
            for lt in range(JT):
                jsl = ts(lt, P)
                psq = pproj.tile([P, TC], F32, tag="proj")
                for d in range(DT):
                    nc.tensor.matmul(
                        out=psq[:], lhsT=wq_sb[:, d, jsl], rhs=xq_c[:, d, :],
                        start=(d == 0), stop=(d == DT - 1),
                    )
                psq_t.append(psq)
            y_t = []
            for lt in range(JT):
                rq = ipool.tile([P, TC], in_dt, tag="rpos")
                nc.scalar.activation(out=rq[:], in_=psq_t[lt][:], func=AF.Relu,
                                     bias=bq_sb[:, lt:lt + 1])
                eq = ipool.tile([P, TC], in_dt, tag="ex")
                nc.scalar.activation(out=eq[:], in_=psq_t[lt][:], func=AF.Exp,
                                     bias=bq_sb[:, lt:lt + 1])
                pq = ipool.tile([P, TC], in_dt, tag="pq")
                nc.vector.scalar_tensor_tensor(
                    out=pq[:], in0=eq[:], scalar=1.0, in1=rq[:],
                    op0=M.min, op1=M.add)
                y = spool.tile([P, TC], in_dt, tag=f"y{lt}")
                nc.vector.tensor_tensor(out=y[:], in0=s_t[lt][:], in1=pq[:], op=M.mult)
                y_t.append(y)

            # O projection for the PREVIOUS chunk: emitted after this
            # chunk's q/k/v matmuls so the PE never stalls waiting for the
            # elementwise chain to produce this chunk's y tiles.
            if y_prev is not None:
                emit_oproj(c - 1, y_prev)
            y_prev = y_t
            x_tiles, x_next = x_next, nxt

        emit_oproj(NCH - 1, y_prev, split_copies=True)

    nc.compile()
    return nc


_NC_CACHE = {}


def _get_nc():
    key = (IN_DT_NAME, N_WARMUP, WG_GPS, TAIL_GPS)
    if key not in _NC_CACHE:
        _NC_CACHE[key] = build_nc()
    return _NC_CACHE[key]


def _np_in_dt():
    if IN_DT_NAME in ("bfloat16", "float16"):
        import ml_dtypes
        return ml_dtypes.bfloat16 if IN_DT_NAME == "bfloat16" else np.float16
    return np.float32


def make_in_maps(query, key, value, beta, Wq, bq, Wk, bk, Wv, bv, Wb, bb, Wo, bo):
    """Host-side shard prep: core_id = b*2 + hg."""
    ndt = _np_in_dt()
    TC = _tc(IN_DT_NAME)
    NCH = S // TC

    def xpack(x):  # [S, H_DIM] -> [p, chunk, dt, t] in in_dt
        a = np.asarray(x, np.float32).T            # [H_DIM, S] = [dt*128+p, c*TC+t]
        a = a.reshape(DT, P, NCH, TC)              # [dt, p, c, t]
        a = a.transpose(1, 2, 0, 3)                # [p, c, dt, t]
        return np.ascontiguousarray(a).astype(ndt)

    def t32(x):
        return np.ascontiguousarray(np.asarray(x, np.float32).T).astype(ndt)

    xqs = [xpack(query[b]) for b in range(B)]
    xks = [xpack(key[b]) for b in range(B)]
    xvs = [xpack(value[b]) for b in range(B)]
    # gate b computed host-side (0.4% of FLOPs), pre-broadcast per lane
    Wbf = np.asarray(Wb, np.float32)
    bbf0 = np.asarray(bb, np.float32)
    z = np.einsum('bsd,hd->bsh', np.asarray(beta, np.float32), Wbf) + bbf0
    bgate = 1.0 / (1.0 + np.exp(-z))                      # [B, S, 16]

    def bpack(bl):  # [S, J] -> [p, chunk, lt, t]
        a = bl.T.reshape(JT, P, NCH, TC)                  # [lt, p, c, t]
        return np.ascontiguousarray(a.transpose(1, 2, 0, 3)).astype(ndt)
    bqf = np.asarray(bq, np.float32)
    bkf = np.asarray(bk, np.float32)
    bvf = np.asarray(bv, np.float32)

    in_maps = []
    for b in range(B):
        for hg in range(HG):
            jsl = slice(hg * J, (hg + 1) * J)
            hsl = slice(hg * HPC, (hg + 1) * HPC)

            def lanes(v):  # [J] -> [128, 4] per lane-tile columns
                return np.ascontiguousarray(v[jsl].reshape(JT, P).T)

            in_maps.append({
                "xq": xqs[b], "xk": xks[b], "xv": xvs[b],
                "bbb": bpack(np.repeat(bgate[b][:, hsl], HEAD_DIM, axis=1)),
                "wq": t32(Wq[jsl]), "wk": t32(Wk[jsl]), "wv": t32(Wv[jsl]),
                "wo": t32(Wo[:, jsl]),
                "bq": lanes(bqf), "bk": lanes(bkf),
                "bv": lanes(bvf),
            })
    return in_maps


LAST_RESULTS = None


def kernel(**inputs):
    global LAST_RESULTS
    nc = _get_nc()
    in_maps = make_in_maps(**inputs)
    res = run_bass_kernel_spmd(nc, in_maps, core_ids=list(range(NCORES)),
                               trace=bool(os.environ.get("DELTA_TRACE")))
    LAST_RESULTS = res
    bo = np.asarray(inputs["bo"], np.float32)
    out = np.empty((B, S, H_DIM), np.float32)
    for b in range(B):
        m = (np.asarray(res.results[2 * b]["out"], np.float32)
             + np.asarray(res.results[2 * b + 1]["out"], np.float32))
        out[b] = m.T + bo
    return out
